# revision 1
# baseline (speedup 1.0000x reference)
"""Trainium2 Bass kernel for DendriticANN (dense_mlp).

Reference computation (fp32):
    h = lrelu(x @ W_in.T + b_in)                        # [B, H]
    for l in 0..L-1:
        dend = lrelu(einsum('bh,ndh->bnd', h, Wd[l]))   # [B, H, D]
        soma = lrelu(einsum('bnd,nd->bn', dend, sd[l])) # [B, H]
        h = lrelu(soma)
    out = h @ W_out.T + b_out                           # [B, OUT]

Strategy: tensor-parallel over the H neuron axis across 8 NeuronCores.
Each core owns 128 neurons; activations live transposed on-chip
(hT = [H partitions, B free]) so every matmul contracts over the
partition dim with no transposes anywhere:

  - input layer (sharded over H): s0_c = lrelu(W_in_c @ x.T + b_in_c)   [128, B]
  - AllGather over cores -> full hT [1024, B]
  - hidden layer per core: for each 128-wide (neuron, dendrite) tile,
      dend^T = WdT_chunk.T @ hT (8 accumulating matmuls, K=128 each)
      s1 = lrelu(dend^T)                                 (ScalarE, alpha=.01)
      soma^T = blockdiag(sd).T @ s1                      (PE does the D-sum,
                                                          sd folded into weights)
      h' = lrelu(lrelu(soma^T)) = lrelu_{1e-4}(soma^T)   (ScalarE, alpha=1e-4)
    -> AllGather -> next hT
  - output layer sharded over OUT rows: outT_c = W_out_c @ hT + b_out_c [125, B]
    (host concatenates the 8 shards; final transpose on host)

Matmuls run in float32r (TF32-like, 1 cyc/row at N>=256 vs 4 for fp32).
"""

import numpy as np

import concourse.bass as bass
import concourse.mybir as mybir
import concourse.tile as tile
from concourse import bacc
from concourse.bass_utils import run_bass_kernel_spmd

# Problem constants (hardcoded per harness contract)
B, IN, H, OUT, L, D = 512, 1024, 1024, 1000, 4, 16
N_CORES = 8
HS = H // N_CORES           # 128 neurons per core
OS = OUT // N_CORES         # 125 output rows per core
KT = H // 128               # 8 k-tiles over the contraction dim
NDT = HS * D // 128         # 16 (neuron,dendrite) tiles of 128 per core
N8 = 128 // D               # 8 neurons per nd-tile

AF = mybir.ActivationFunctionType
F32 = mybir.dt.float32

# matmul dtype: "fp32r" (TF32-like, full speed), "fp32" (exact, 4x slower),
# "bf16" (full speed, half the DMA bytes, lower precision)
MM_DT = "fp32r"

# wd streaming chunk pool depth ([128, 1024] tiles; 4KB/partition in fp32)
WD_BUFS = 16


def _sdt(mm_dt):
    return {
        "fp32r": mybir.dt.float32r,
        "fp32": mybir.dt.float32,
        "bf16": mybir.dt.bfloat16,
    }[mm_dt]


def build_module(mm_dt=None, wd_bufs=None, reps=1, ablate=()):
    """Build + compile the SPMD Bass module. Returns nc.

    reps > 1 unrolls the whole pipeline R times inside one NEFF — used by
    test.py to measure steady-state per-iteration device time via the
    slope between rep counts (no NTFF profiling available under axon).

    ablate: set of {"noag", "nosoma", "noact", "nowd"} — timing-only
    variants that produce WRONG results but isolate stage costs.
    """
    if mm_dt is None:
        mm_dt = MM_DT
    if wd_bufs is None:
        wd_bufs = WD_BUFS
    ablate = set(ablate)
    sdt = _sdt(mm_dt)
    nc = bacc.Bacc("TRN2", target_bir_lowering=False, debug=False,
                   num_devices=N_CORES)

    # ---- DRAM I/O (per-core shards, host-prepared layouts) ----
    xT_d = nc.dram_tensor("xT", [128, KT, B], sdt, kind="ExternalInput").ap()
    winT_d = nc.dram_tensor("winT", [128, KT, H], sdt, kind="ExternalInput").ap()
    bin_d = nc.dram_tensor("b_in", [128, KT], F32, kind="ExternalInput").ap()
    wd_d = nc.dram_tensor("wdT", [L, NDT, 128, KT * 128], sdt,
                          kind="ExternalInput").ap()
    sdb_d = nc.dram_tensor("sdb", [128, L * NDT * N8], sdt,
                           kind="ExternalInput").ap()
    woutT_d = nc.dram_tensor("woutT", [128, KT, OS], sdt,
                             kind="ExternalInput").ap()
    bout_d = nc.dram_tensor("b_out", [OS, 1], F32, kind="ExternalInput").ap()
    outT_d = nc.dram_tensor("outT", [OS, B], F32, kind="ExternalOutput").ap()

    rg = [list(range(N_CORES))]

    with tile.TileContext(nc) as tc:
        with (
            tc.tile_pool(name="const", bufs=1) as cpool,
            tc.tile_pool(name="wd", bufs=wd_bufs) as wdpool,
            tc.tile_pool(name="h", bufs=2) as hpool,
            tc.tile_pool(name="s1p", bufs=4) as s1pool,
            tc.tile_pool(name="soma", bufs=2) as spool,
            tc.tile_pool(name="outp", bufs=1) as opool,
            tc.tile_pool(name="psd", bufs=3, space="PSUM") as ppd,
            tc.tile_pool(name="pss", bufs=3, space="PSUM") as pps,
            tc.tile_pool(name="dram", bufs=2, space="DRAM") as dpool,
        ):
            # ---- persistent loads ----
            xT = cpool.tile([128, KT, B], sdt, name="xT_sb")
            nc.sync.dma_start(xT[:], xT_d[:])
            winT = cpool.tile([128, KT, H], sdt, name="winT_sb")
            nc.sync.dma_start(winT[:], winT_d[:])
            b_in = cpool.tile([128, KT], F32, name="bin_sb")
            nc.sync.dma_start(b_in[:], bin_d[:])
            sdb = cpool.tile([128, L * NDT * N8], sdt, name="sdb_sb")
            nc.sync.dma_start(sdb[:], sdb_d[:])
            woutT = cpool.tile([128, KT, OS], sdt, name="woutT_sb")
            nc.sync.dma_start(woutT[:], woutT_d[:])
            b_out = cpool.tile([OS, 1], F32, name="bout_sb")
            nc.sync.dma_start(b_out[:], bout_d[:])
            if "nowd" in ablate:
                wd_warm = cpool.tile([128, KT * 128], sdt, name="wd_warm")
                nc.sync.dma_start(wd_warm[:], wd_d[0, 0])

            def gather(agin):
                """AllGather [128,B] core shards -> full hT [128, KT, B] in SBUF."""
                hT = hpool.tile([128, KT, B], sdt, tag="hT", name="hT")
                if "noag" in ablate:
                    # timing ablation: skip the collective, replicate own shard
                    for kt in range(KT):
                        nc.sync.dma_start(hT[:, kt, :], agin[:])
                    return hT
                agout = dpool.tile([H, B], sdt, addr_space="Shared",
                                   tag="agout", name="agout")
                nc.gpsimd.collective_compute(
                    "AllGather",
                    mybir.AluOpType.bypass,
                    replica_groups=rg,
                    ins=[agin[:].opt()],
                    outs=[agout[:].opt()],
                )
                gv = agout[:].rearrange("(kt k) b -> k kt b", k=128)
                nc.sync.dma_start(hT[:], gv)
                return hT

            def one_pass():
                # ---- input layer: full H on every core (redundant compute
                # beats paying an extra AllGather) ----
                hT = hpool.tile([128, KT, B], sdt, tag="hT", name="hT0")
                for mt in range(KT):
                    ps0 = ppd.tile([128, B], F32, tag="pd", name=f"ps0_m{mt}")
                    for kt in range(KT):
                        nc.tensor.matmul(
                            ps0[:], winT[:, kt, mt * 128:(mt + 1) * 128],
                            xT[:, kt, :],
                            start=(kt == 0), stop=(kt == KT - 1))
                    nc.scalar.activation(hT[:, mt, :], ps0[:], AF.Lrelu,
                                         bias=b_in[:, mt:mt + 1], alpha=0.01)

                # ---- hidden layers ----
                for l in range(L):
                    agin = dpool.tile([HS, B], sdt, tag="agin",
                                      name=f"agin_l{l}")
                    # 16 soma chunks packed as 4 partition-groups (ACT can
                    # write at partition bases 0/32/64/96) x 4 free slots
                    somaS = spool.tile([128, NDT // 4, B], sdt, tag="soma",
                                       name=f"somaS_l{l}")
                    for t in range(NDT):
                        if "nowd" in ablate:
                            wd_chunk = wd_warm
                        else:
                            wd_chunk = wdpool.tile([128, KT * 128], sdt,
                                                   tag="wd",
                                                   name=f"wd_l{l}_t{t}")
                            nc.sync.dma_start(wd_chunk[:], wd_d[l, t])
                        psd = ppd.tile([128, B], F32, tag="pd",
                                       name=f"pd_l{l}_t{t}")
                        for kt in range(KT):
                            nc.tensor.matmul(
                                psd[:],
                                wd_chunk[:, kt * 128:(kt + 1) * 128],
                                hT[:, kt, :],
                                start=(kt == 0), stop=(kt == KT - 1),
                            )
                        if "noact" in ablate:
                            if t == NDT - 1:
                                s1 = s1pool.tile([128, B], sdt, tag="s1",
                                                 name=f"s1_l{l}_t{t}")
                                nc.vector.tensor_copy(s1[:], psd[:])
                                nc.sync.dma_start(agin[:, :], s1[:])
                            continue
                        s1 = s1pool.tile([128, B], sdt, tag="s1",
                                         name=f"s1_l{l}_t{t}")
                        nc.scalar.activation(s1[:], psd[:], AF.Lrelu,
                                             alpha=0.01)
                        if "nosoma" in ablate:
                            if t == NDT - 1:
                                nc.sync.dma_start(agin[:, :], s1[:])
                            continue
                        pss_t = pps.tile([N8, B], F32, tag="ps",
                                         name=f"ps_l{l}_t{t}")
                        off = (l * NDT + t) * N8
                        nc.tensor.matmul(pss_t[:], sdb[:, off:off + N8],
                                         s1[:], start=True, stop=True)
                        if "nos2" in ablate:
                            if t == NDT - 1:
                                nc.sync.dma_start(agin[:, :], s1[:])
                            continue
                        # h' = lrelu(lrelu(soma)) = lrelu_{1e-4}(soma).
                        # NB: a second Lrelu table with a different alpha
                        # silently aliases the first, but Prelu gets its own
                        # table -> single fused op is safe.
                        g, f = t % 4, t // 4
                        nc.scalar.activation(
                            somaS[32 * g:32 * g + N8, f, :], pss_t[:],
                            AF.Prelu, alpha=1e-4)
                    # agin row t*8+p = 32f+8g+p ; somaS partition 32g+p
                    sv = agin[:].rearrange("(f g p) b -> g p f b",
                                           f=NDT // 4, g=4)
                    for g in range(4):
                        nc.sync.dma_start(sv[g],
                                          somaS[32 * g:32 * g + N8, :, :])
                    hT = gather(agin)

                # ---- output layer (OUT-sharded) ----
                pso = ppd.tile([OS, B], F32, tag="pd", name="pso")
                for kt in range(KT):
                    nc.tensor.matmul(pso[:], woutT[:, kt, :], hT[:, kt, :],
                                     start=(kt == 0), stop=(kt == KT - 1))
                out_sb = opool.tile([OS, B], F32, name="out_sb")
                nc.scalar.activation(out_sb[:], pso[:], AF.Identity,
                                     bias=b_out[:])
                nc.sync.dma_start(outT_d[:], out_sb[:])

            for _rep in range(reps):
                one_pass()

    nc.compile()
    return nc


def _np_dt(mm_dt):
    if mm_dt == "bf16":
        import ml_dtypes
        return np.dtype(ml_dtypes.bfloat16)
    return np.dtype(np.float32)


def make_in_maps(x, W_in, b_in, Wd, sd, W_out, b_out, mm_dt=MM_DT):
    """Host-side sharding/layout prep. Returns per-core input dicts."""
    ndt = _np_dt(mm_dt)
    f32 = np.float32
    x = np.asarray(x, f32)
    W_in = np.asarray(W_in, f32)
    b_in = np.asarray(b_in, f32)
    Wd = np.asarray(Wd, f32)
    sd = np.asarray(sd, f32)
    W_out = np.asarray(W_out, f32)
    b_out = np.asarray(b_out, f32)

    # xT: [k, kt, b] (shared by all cores)
    xT = np.ascontiguousarray(x.reshape(B, KT, 128).transpose(2, 1, 0)).astype(ndt)
    # winT: [k, kt, m] over the FULL H (input layer is computed redundantly)
    winT = np.ascontiguousarray(
        W_in.reshape(H, KT, 128).transpose(2, 1, 0)).astype(ndt)
    bin_full = np.ascontiguousarray(b_in.reshape(KT, 128).T)

    in_maps = []
    for c in range(N_CORES):

        Wd_c = Wd[:, c * HS:(c + 1) * HS, :, :]                # [L, 128, D, H]
        wdT = np.ascontiguousarray(
            Wd_c.reshape(L, NDT, N8, D, KT, 128).transpose(0, 1, 5, 4, 2, 3)
        ).reshape(L, NDT, 128, KT * 128).astype(ndt)

        sd_c = sd[:, c * HS:(c + 1) * HS, :]                   # [L, 128, D]
        sd_r = sd_c.reshape(L, NDT, N8, D)                     # [l, t, m, d]
        sdb = np.zeros((128, L, NDT, N8), f32)
        for m in range(N8):
            # partition nd = m*D + d gets sd of neuron m in each tile
            sdb[m * D:(m + 1) * D, :, :, m] = sd_r[:, :, m, :].transpose(2, 0, 1)
        sdb = np.ascontiguousarray(sdb.reshape(128, L * NDT * N8)).astype(ndt)

        Wo = W_out[c * OS:(c + 1) * OS, :]                     # [125, H]
        woutT = np.ascontiguousarray(
            Wo.reshape(OS, KT, 128).transpose(2, 1, 0)).astype(ndt)
        bout_c = np.ascontiguousarray(b_out[c * OS:(c + 1) * OS, None])

        in_maps.append({
            "xT": xT,
            "winT": winT,
            "b_in": bin_full,
            "wdT": wdT,
            "sdb": sdb,
            "woutT": woutT,
            "b_out": bout_c,
        })
    return in_maps


_CACHE = {}


def get_module(mm_dt=None, wd_bufs=None):
    if mm_dt is None:
        mm_dt = MM_DT
    if wd_bufs is None:
        wd_bufs = WD_BUFS
    key = (mm_dt, wd_bufs)
    if key not in _CACHE:
        _CACHE[key] = build_module(mm_dt, wd_bufs)
    return _CACHE[key]


def kernel(x, W_in, b_in, Wd, sd, W_out, b_out):
    """Full-input -> full-output entry point (harness contract)."""
    nc = get_module()
    in_maps = make_in_maps(x, W_in, b_in, Wd, sd, W_out, b_out, MM_DT)
    res = run_bass_kernel_spmd(nc, in_maps, core_ids=list(range(N_CORES)))
    out = np.concatenate([res.results[c]["outT"].T for c in range(N_CORES)],
                         axis=1)
    return np.ascontiguousarray(out.astype(np.float32))



# revision 2
# speedup vs baseline: 16.8887x; 16.8887x over previous
"""Trainium2 Bass kernel for DendriticANN (dense_mlp) — v2.

Reference computation (fp32):
    h = lrelu(x @ W_in.T + b_in)                        # [B, H]
    for l in 0..L-1:
        dend = lrelu(einsum('bh,ndh->bnd', h, Wd[l]))   # [B, H, D]
        soma = lrelu(einsum('bnd,nd->bn', dend, sd[l])) # [B, H]
        h = lrelu(soma)
    out = h @ W_out.T + b_out                           # [B, OUT]

Strategy (v2): tensor-parallel over H (128 neurons/core), fp16 operands,
all weights RESIDENT in SBUF (Wd fp16 = 16.8 MB/core fits), and the batch
split in two halves so each layer's AllGather overlaps the other half's
compute:

    in(b0) in(b1) | X(0,b0) X(0,b1) | X(1,b0) ... | out(b0) out(b1)
                        (AG(0,b0) runs during X(0,b1))

Per (layer, half) X: 16 (neuron,dendrite) tiles of 8 accumulating fp16
matmuls (K=128, N=256) + Lrelu; somas ride the PE as 4 column-tiled
matmuls (tile_position=(0,32g), M=32 zero-padded) into one PSUM tile so a
single full-width Prelu(alpha=1e-4) finishes the layer.
"""

import numpy as np

import concourse.bass as bass
import concourse.mybir as mybir
import concourse.tile as tile
from concourse import bacc
from concourse.bass_utils import run_bass_kernel_spmd

# Problem constants (hardcoded per harness contract)
B, IN, H, OUT, L, D = 512, 1024, 1024, 1000, 4, 16
N_CORES = 8
HS = H // N_CORES           # 128 neurons per core
OS = OUT // N_CORES         # 125 output rows per core
KT = H // 128               # 8 k-tiles over the contraction dim
NDT = HS * D // 128         # 16 (neuron,dendrite) tiles of 128 per core
N8 = 128 // D               # 8 neurons per nd-tile
NG = 4                      # soma col-tile groups per f-block
FB = NDT // NG              # 4 f-blocks per layer
NH = 2                      # batch halves
BH = B // NH                # 256

AF = mybir.ActivationFunctionType
F32 = mybir.dt.float32

MM_DT = "fp16"


def _sdt(mm_dt):
    return {
        "fp16": mybir.dt.float16,
        "bf16": mybir.dt.bfloat16,
    }[mm_dt]


def build_module(mm_dt=None, reps=1, ablate=()):
    """Build + compile the SPMD Bass module. Returns nc.

    reps > 1 unrolls the pipeline R times inside one NEFF for slope timing.
    ablate: {"noag", "nosoma", "noact"} — timing-only wrong-result variants.
    """
    if mm_dt is None:
        mm_dt = MM_DT
    ablate = set(ablate)
    sdt = _sdt(mm_dt)
    nc = bacc.Bacc("TRN2", target_bir_lowering=False, debug=False,
                   num_devices=N_CORES)

    # ---- DRAM I/O (per-core shards, host-prepared layouts) ----
    xT_d = nc.dram_tensor("xT", [128, KT, B], sdt, kind="ExternalInput").ap()
    winT_d = nc.dram_tensor("winT", [128, KT, H], sdt, kind="ExternalInput").ap()
    bin_d = nc.dram_tensor("b_in", [128, KT], F32, kind="ExternalInput").ap()
    wd_d = nc.dram_tensor("wdT", [128, L * NDT, KT * 128], sdt,
                          kind="ExternalInput").ap()
    sdb_d = nc.dram_tensor("sdb", [128, L * NDT * 32], sdt,
                           kind="ExternalInput").ap()
    woutT_d = nc.dram_tensor("woutT", [128, KT, OS], sdt,
                             kind="ExternalInput").ap()
    bout_d = nc.dram_tensor("b_out", [1, OS], sdt, kind="ExternalInput").ap()
    outT_d = nc.dram_tensor("outT", [OS, B], F32, kind="ExternalOutput").ap()

    rg = [list(range(N_CORES))]

    with tile.TileContext(nc) as tc:
        with (
            tc.tile_pool(name="const", bufs=1) as cpool,
            tc.tile_pool(name="h", bufs=2) as hpool,
            tc.tile_pool(name="s1p", bufs=6) as s1pool,
            tc.tile_pool(name="soma", bufs=2) as spool,
            tc.tile_pool(name="outp", bufs=2) as opool,
            tc.tile_pool(name="psd", bufs=5, space="PSUM") as ppd,
            tc.tile_pool(name="pss", bufs=2, space="PSUM") as pps,
            tc.tile_pool(name="dram", bufs=3, space="DRAM") as dpool,
        ):
            # ---- persistent loads (once per NEFF execution) ----
            xT = cpool.tile([128, KT, B], sdt, name="xT_sb")
            nc.sync.dma_start(xT[:], xT_d[:])
            winT = cpool.tile([128, KT, H], sdt, name="winT_sb")
            nc.sync.dma_start(winT[:], winT_d[:])
            b_in = cpool.tile([128, KT], F32, name="bin_sb")
            nc.sync.dma_start(b_in[:], bin_d[:])
            wdR = cpool.tile([128, L * NDT, KT * 128], sdt, name="wd_sb")
            nc.sync.dma_start(wdR[:], wd_d[:])
            sdb = cpool.tile([128, L * NDT * 32], sdt, name="sdb_sb")
            nc.sync.dma_start(sdb[:], sdb_d[:])
            woutT = cpool.tile([128, KT, OS], sdt, name="woutT_sb")
            nc.sync.dma_start(woutT[:], woutT_d[:])
            b_out = cpool.tile([1, OS], sdt, name="bout_sb")
            nc.sync.dma_start(b_out[:], bout_d[:])
            ones = cpool.tile([1, B], sdt, name="ones_sb")
            nc.vector.memset(ones[:], 1.0)

            def gather(agin, h, l):
                """AllGather [128,BH] shard -> full hT [128,KT,BH] in SBUF."""
                hT = hpool.tile([128, KT, BH], sdt, tag=f"hT{h}",
                                name=f"hT_l{l}h{h}")
                if "noag" in ablate:
                    for kt in range(KT):
                        nc.sync.dma_start(hT[:, kt, :], agin[:])
                    return hT
                agout = dpool.tile([H, BH], sdt, addr_space="Shared",
                                   tag=f"agout{h}", name=f"agout_l{l}h{h}")
                nc.gpsimd.collective_compute(
                    "AllGather",
                    mybir.AluOpType.bypass,
                    replica_groups=rg,
                    ins=[agin[:].opt()],
                    outs=[agout[:].opt()],
                )
                # copyback on SP, which carries nothing else time-critical:
                # its sem-wait on the collective must not block the next
                # gather's staging
                gv = agout[:].rearrange("(kt k) b -> k kt b", k=128)
                nc.sync.dma_start(hT[:], gv)
                return hT

            def output_layer(hTs):
                # ---- output layer (OUT-sharded), per half ----
                for h in range(NH):
                    pso = pps.tile([OS, BH], F32, tag="ps", name=f"pso{h}")
                    # bias as a K=1 matmul so the ACT engine never needs a
                    # non-Lrelu table
                    nc.tensor.matmul(pso[:], b_out[:],
                                     ones[:, h * BH:(h + 1) * BH],
                                     start=True, stop=False)
                    for kt in range(KT):
                        nc.tensor.matmul(pso[:], woutT[:, kt, :],
                                         hTs[h][:, kt, :],
                                         start=False, stop=(kt == KT - 1))
                    out_sb = opool.tile([OS, BH], F32, tag="out",
                                        name=f"out_sb{h}")
                    nc.vector.tensor_copy(out_sb[:], pso[:])
                    nc.sync.dma_start(outT_d[:, h * BH:(h + 1) * BH],
                                      out_sb[:])

            def one_pass(prev_hTs):
                # ---- input layer: full H on every core, per half ----
                hTs = []
                for h in range(NH):
                    hT = hpool.tile([128, KT, BH], sdt, tag=f"hT{h}",
                                    name=f"hT0_h{h}")
                    for mt in range(KT):
                        ps0 = ppd.tile([128, BH], F32, tag="pd",
                                       name=f"ps0_m{mt}h{h}")
                        for kt in range(KT):
                            nc.tensor.matmul(
                                ps0[:], winT[:, kt, mt * 128:(mt + 1) * 128],
                                xT[:, kt, h * BH:(h + 1) * BH],
                                start=(kt == 0), stop=(kt == KT - 1))
                        nc.scalar.activation(hT[:, mt, :], ps0[:], AF.Lrelu,
                                             bias=b_in[:, mt:mt + 1],
                                             alpha=0.01)
                    hTs.append(hT)

                # ---- previous rep's output layer: deferred to here so its
                # last AllGather hides under the input-layer matmuls ----
                if prev_hTs is not None:
                    output_layer(prev_hTs)

                # ---- hidden layers, halves pipelined around the AllGather --
                for l in range(L):
                    newh = []
                    for h in range(NH):
                        hT = hTs[h]
                        agin = dpool.tile([HS, BH], sdt, tag=f"agin{h}",
                                          name=f"agin_l{l}h{h}")
                        somaS = spool.tile([128, FB, BH], sdt, tag=f"soma{h}",
                                           name=f"somaS_l{l}h{h}")
                        # nd-tiles run in PAIRS sharing one PSUM bank, one
                        # Lrelu per pair; soma blocks (4 col-tiled matmuls)
                        # are issued a full pair later so the PE FIFO never
                        # waits on ACT.
                        pend = {}

                        def issue_block(f):
                            pss = pps.tile([128, BH], F32, tag="ps",
                                           name=f"ps_l{l}h{h}f{f}")
                            for g in range(NG):
                                t = NG * f + g
                                s1D = pend[t // 2]
                                off = (l * NDT + t) * 32
                                # sdb cols 8..31 are zero so all 32 psum
                                # partitions are written (one full-width
                                # Prelu per f-block below)
                                nc.tensor.matmul(
                                    pss[32 * g:32 * g + 32, :],
                                    sdb[:, off:off + 32],
                                    s1D[:, t % 2, :],
                                    start=True, stop=True,
                                    tile_position=(0, 32 * g))
                            pend.pop(NG * f // 2, None)
                            pend.pop(NG * f // 2 + 1, None)
                            # h' = lrelu(lrelu(soma)) = lrelu_1e-4
                            nc.scalar.activation(somaS[:, f, :], pss[:],
                                                 AF.Prelu, alpha=1e-4)

                        for j in range(NDT // 2):
                            psdD = ppd.tile([128, 2, BH], F32, tag="pd",
                                            name=f"pd_l{l}h{h}j{j}")
                            for u in range(2):
                                t = 2 * j + u
                                for kt in range(KT):
                                    nc.tensor.matmul(
                                        psdD[:, u, :],
                                        wdR[:, l * NDT + t,
                                            kt * 128:(kt + 1) * 128],
                                        hT[:, kt, :],
                                        start=(kt == 0), stop=(kt == KT - 1))
                            if "noact" in ablate:
                                continue
                            s1D = s1pool.tile([128, 2, BH], sdt, tag="s1",
                                              name=f"s1_l{l}h{h}j{j}")
                            nc.scalar.activation(s1D[:], psdD[:], AF.Lrelu,
                                                 alpha=0.01)
                            if "nosoma" in ablate:
                                continue
                            pend[j] = s1D
                            if j >= 2 and j % 2 == 0:
                                issue_block((j - 2) // 2)
                        if "noact" not in ablate and "nosoma" not in ablate:
                            issue_block(FB - 1)
                        if "noact" in ablate or "nosoma" in ablate:
                            # keep the dataflow alive for timing ablations
                            nc.gpsimd.dma_start(agin[:], hT[:, 0, :])
                        else:
                            # agin row 8t+p = 32f+8g+p ; somaS part 32g+p.
                            # Staged on the ACT queue right after the last
                            # Prelu: fires ASAP, no cross-engine hop, and
                            # never blocked by other queues' sem-waits.
                            sv = agin[:].rearrange("(f g p) b -> g p f b",
                                                   f=FB, g=NG)
                            for g in range(NG):
                                nc.scalar.dma_start(
                                    sv[g], somaS[32 * g:32 * g + N8, :, :])
                        newh.append(gather(agin, h, l))
                    hTs = newh
                return hTs

            prev = None
            for _rep in range(reps):
                prev = one_pass(prev)
            output_layer(prev)

    nc.compile()
    return nc


def _np_dt(mm_dt):
    import ml_dtypes
    return {
        "fp16": np.dtype(np.float16),
        "bf16": np.dtype(ml_dtypes.bfloat16),
    }[mm_dt]


def make_in_maps(x, W_in, b_in, Wd, sd, W_out, b_out, mm_dt=None):
    """Host-side sharding/layout prep. Returns per-core input dicts."""
    if mm_dt is None:
        mm_dt = MM_DT
    ndt = _np_dt(mm_dt)
    f32 = np.float32
    x = np.asarray(x, f32)
    W_in = np.asarray(W_in, f32)
    b_in = np.asarray(b_in, f32)
    Wd = np.asarray(Wd, f32)
    sd = np.asarray(sd, f32)
    W_out = np.asarray(W_out, f32)
    b_out = np.asarray(b_out, f32)

    # xT: [k, kt, b] (shared by all cores)
    xT = np.ascontiguousarray(
        x.reshape(B, KT, 128).transpose(2, 1, 0)).astype(ndt)
    # winT: [k, kt, m] over the FULL H (input layer computed redundantly)
    winT = np.ascontiguousarray(
        W_in.reshape(H, KT, 128).transpose(2, 1, 0)).astype(ndt)
    bin_full = np.ascontiguousarray(b_in.reshape(KT, 128).T)

    in_maps = []
    for c in range(N_CORES):
        Wd_c = Wd[:, c * HS:(c + 1) * HS, :, :]                # [L, 128, D, H]
        # [l, t, m, d, kt, k] -> [k, l*NDT+t, kt*128 + m*D + d]
        wdT = np.ascontiguousarray(
            Wd_c.reshape(L, NDT, N8, D, KT, 128).transpose(5, 0, 1, 4, 2, 3)
        ).reshape(128, L * NDT, KT * 128).astype(ndt)

        sd_c = sd[:, c * HS:(c + 1) * HS, :]                   # [L, 128, D]
        sd_r = sd_c.reshape(L, NDT, N8, D)                     # [l, t, m, d]
        sdb = np.zeros((128, L, NDT, 32), f32)
        for m in range(N8):
            # partition nd = m*D + d gets sd of neuron m in each tile
            sdb[m * D:(m + 1) * D, :, :, m] = sd_r[:, :, m, :].transpose(2, 0, 1)
        sdb = np.ascontiguousarray(sdb.reshape(128, L * NDT * 32)).astype(ndt)

        Wo = W_out[c * OS:(c + 1) * OS, :]                     # [125, H]
        woutT = np.ascontiguousarray(
            Wo.reshape(OS, KT, 128).transpose(2, 1, 0)).astype(ndt)
        bout_c = np.ascontiguousarray(b_out[None, c * OS:(c + 1) * OS]).astype(ndt)

        in_maps.append({
            "xT": xT,
            "winT": winT,
            "b_in": bin_full,
            "wdT": wdT,
            "sdb": sdb,
            "woutT": woutT,
            "b_out": bout_c,
        })
    return in_maps


_CACHE = {}


def get_module(mm_dt=None):
    if mm_dt is None:
        mm_dt = MM_DT
    if mm_dt not in _CACHE:
        _CACHE[mm_dt] = build_module(mm_dt)
    return _CACHE[mm_dt]


def kernel(x, W_in, b_in, Wd, sd, W_out, b_out):
    """Full-input -> full-output entry point (harness contract)."""
    nc = get_module()
    in_maps = make_in_maps(x, W_in, b_in, Wd, sd, W_out, b_out, MM_DT)
    res = run_bass_kernel_spmd(nc, in_maps, core_ids=list(range(N_CORES)))
    out = np.concatenate([res.results[c]["outT"].T for c in range(N_CORES)],
                         axis=1)
    return np.ascontiguousarray(out.astype(np.float32))
